# revision 5
# baseline (speedup 1.0000x reference)
import numpy as np
import jax
import jax.numpy as jnp

# nn_DPSTCN: hardcoded problem shapes
B, N, L, D, H, GOUT = 256, 307, 12, 16, 8, 32
hd = D // H
M = 8           # cores
BC = B // M     # 32 batches per core

f32 = jnp.float32


def _pos_encoding():
    pos = np.arange(L, dtype=np.float32)[:, None]
    div = np.power(10000.0, np.arange(0, D, 2, dtype=np.float32) / D)
    ang = pos / div
    P = np.zeros((L, D), dtype=np.float32)
    P[:, 0::2] = np.sin(ang)
    P[:, 1::2] = np.cos(ang)
    return P  # [L, D]


def _core_fn(fx16, te16, his16, adj8, pe,
             Wq, bq, Wk, bk, Wv, bv, Wo, bo, Wg, Wt, bg, W1, b1, W2, b2):
    # fx16: [BC, N, L] fp16 shard; te16: [BC, L, D] fp16 (host-gathered
    # day_emb[day_cyc]+week_emb[week_cyc]); his16: [N, 11+B] fp16 replicated
    # (host all-gather of last timesteps per the sharding hint); adj8 uint8.
    # Everything vertex-indexed is kept n-major [N, BC, ...] so the A@ matmuls
    # and the n-batched MLP read their operands without layout changes.
    f = jnp.transpose(fx16.astype(f32), (1, 0, 2))         # [N, BC, L]
    te = te16.astype(f32)
    his = his16.astype(f32)
    adj = adj8.astype(f32)

    # dynamic graph from the full batch window
    sqn = jnp.sum(his * his, axis=1)
    d2 = sqn[:, None] + sqn[None, :] - 2.0 * (his @ his.T)
    fun = jnp.sqrt(jnp.maximum(d2, 0.0))                   # [N, N]
    A_dyn = jax.nn.softmax(-fun, axis=-1)                  # [N, N]
    A_st = adj / (jnp.sum(adj, axis=-1, keepdims=True) + 1.0)

    # x_t = f[n,b,l] + c[b,l,d] with c independent of n -> attention
    # decomposes into per-(b,l) tensors + the per-token 12-vector f.
    c = pe[None] + te                                      # [BC, L, D]
    ones = jnp.ones((D,), f32)
    sq_ = (ones @ Wq).reshape(H, hd)                       # colsum(Wq) per head
    sk_ = (ones @ Wk).reshape(H, hd)
    sv_ = (ones @ Wv).reshape(H, hd)
    cq = (c @ Wq + bq).reshape(BC, L, H, hd)
    ck = (c @ Wk + bk).reshape(BC, L, H, hd)
    cv = (c @ Wv + bv).reshape(BC, L, H, hd)

    g_h = jnp.sum(sq_ * sk_, axis=-1)                      # [H]
    alpha = jnp.einsum('hd,bmhd->bmh', sq_, ck)            # [BC, L(m), H]
    beta = jnp.einsum('blhd,hd->blh', cq, sk_)             # [BC, L(l), H]
    gam = jnp.einsum('blhd,bmhd->bhlm', cq, ck)            # [BC, H, L, L]
    al = jnp.moveaxis(alpha, (1, 2), (2, 1))               # [BC, H, M]
    be = jnp.moveaxis(beta, (1, 2), (2, 1))                # [BC, H, L]
    cvm = jnp.moveaxis(cv, (1, 2), (2, 1))                 # [BC, H, M, hd]

    inv_sqrt = f32(1.0 / np.sqrt(hd))
    # logits[n,b,h,l,m] — broadcasts only, no batched matmuls
    lg = (f[:, :, None, :, None] * f[:, :, None, None, :] * g_h[None, None, :, None, None]
          + f[:, :, None, :, None] * al[None, :, :, None, :]
          + f[:, :, None, None, :] * be[None, :, :, :, None]
          + gam[None]) * inv_sqrt                          # [N, BC, H, L, M]
    ex = jnp.exp(lg)                                       # logits tiny; no max-sub
    s = jnp.sum(ex, axis=-1)                               # [N, BC, H, L]
    P1 = jnp.sum(ex * f[:, :, None, None, :], axis=-1)     # [N, BC, H, L]
    P2a = jnp.sum(ex * cvm[None, :, :, None, :, 0], axis=-1)
    P2b = jnp.sum(ex * cvm[None, :, :, None, :, 1], axis=-1)
    att = jnp.stack([P1 * sv_[:, 0][None, None, :, None] + P2a,
                     P1 * sv_[:, 1][None, None, :, None] + P2b],
                    axis=-1) / s[..., None]                # [N, BC, H, L, hd]
    att = jnp.moveaxis(att, 2, 3).reshape(N, BC, L, D)
    attWo = att @ Wo + bo                                  # [N, BC, L, D]

    # graph mixing: x_tcn = f + c + attWo; A_st@x_tcn collapses to
    # (A_st@f) + rowsum(A_st)*c + A_st@attWo; then @Wt distributes.
    A2 = jnp.concatenate([A_dyn, A_st], axis=0)            # [2N, N]
    Y = A2 @ f.reshape(N, BC * L)
    Y1 = Y[:N].reshape(N, BC, L)                           # A_dyn@f
    Y2 = Y[N:].reshape(N, BC, L)                           # A_st@f
    Z = (A_st @ attWo.reshape(N, BC * L * D)).reshape(N, BC, L, D)

    rsum = jnp.sum(A_st, axis=-1)                          # [N]
    st = jnp.sum(Wt, axis=0)                               # colsum(Wt) [GOUT]
    cWt = c @ Wt                                           # [BC, L, GOUT]

    hid = jax.nn.relu(
        Y1[..., None] * Wg[0]
        + Y2[..., None] * st
        + rsum[:, None, None, None] * cWt[None]
        + Z @ Wt
        + bg)                                              # [N, BC, L, GOUT]

    # per-vertex MLPs: batch dim n leads both operands — no transpose needed
    h1 = jax.nn.relu(jnp.einsum('nblc,nco->nblo', hid, W1.astype(f32))
                     + b1[:, None, None])
    out = jnp.sum(h1 * W2[:, None, None, :, 0], axis=-1) + b2[:, None, None, 0]
    return out.astype(jnp.float16)                         # [N, BC, L]


_pmapped = None


def _get_pmapped():
    global _pmapped
    if _pmapped is None:
        in_axes = (0, 0) + (None,) * 18
        _pmapped = jax.pmap(_core_fn, in_axes=in_axes,
                            devices=jax.devices()[:M])
    return _pmapped


def kernel(flow_x, day_cyc, week_cyc, adj, day_emb, week_emb,
           Wq, bq, Wk, bk, Wv, bv, Wo, bo, Wg, Wt, bg, W1, b1, W2, b2):
    fx = np.asarray(flow_x, dtype=np.float32)
    day_i = np.asarray(day_cyc).astype(np.int32)
    week_i = np.asarray(week_cyc).astype(np.int32)

    # Host side: data movement only — fp16 casts, index gathers, the his
    # window concat (all-gather of last timesteps), and batch sharding.
    fx16 = fx.astype(np.float16)
    his16 = np.concatenate([fx16[0], fx16[1:, :, -1].T], axis=1)  # [N, 11+B]
    te16 = (np.asarray(day_emb, dtype=np.float32)[day_i]
            + np.asarray(week_emb, dtype=np.float32)[week_i]).astype(np.float16)
    adj8 = (np.asarray(adj) != 0).astype(np.uint8)
    pe = _pos_encoding()

    g32 = lambda x: np.asarray(x, dtype=np.float32)
    args = (fx16.reshape(M, BC, N, L), te16.reshape(M, BC, L, D),
            his16, adj8, pe,
            g32(Wq), g32(bq), g32(Wk), g32(bk), g32(Wv), g32(bv),
            g32(Wo), g32(bo), g32(Wg), g32(Wt), g32(bg),
            g32(W1).astype(np.float16), g32(b1), g32(W2), g32(b2))
    out = _get_pmapped()(*args)                            # [M, N, BC, L] fp16
    out = np.asarray(out)
    return np.transpose(out, (0, 2, 1, 3)).reshape(B, N, L).astype(np.float32)
